# revision 37
# baseline (speedup 1.0000x reference)
"""Causal ALiBi sliding-window GQA attention block on 8 TRN2 NeuronCores.

Sharding: 2-way data parallel (batch) x 4-way tensor parallel (heads).
Core c handles batch b = c//4 and query heads [8*(c%4), 8*(c%4)+8)
(= kv heads [2*(c%4), 2*(c%4)+2)).  Each core computes its slice of the
QKV projections, windowed-causal ALiBi attention for its 8 heads, and a
partial output projection; the host sums the 4 TP partials per batch.

Kernel math layout (per core):
  - everything is computed transposed: xT [D,S] streams as the moving
    operand, qT/kT are built with head-dim on partitions so attention
    scores come out as sT[j,i] (j on partitions).
  - ALiBi bias is fused into the score matmul as 2 extra contraction
    rows: k-side aug rows [j; 1], q-side aug rows [slope/SCALE;
    -slope/SCALE*i], so PSUM = qk + bias/SCALE and a single scale-only
    Exp activation produces the (unnormalized) softmax weights.
  - causal-diagonal and window-edge masks are applied by the PE itself:
    an extra matmul (identity stationary x const -30000 tile) adds a
    large negative into the masked positions of the score PSUM, so the
    Exp flushes them to exactly 0.  No vector-engine masking.
  - softmax denominator comes from a ones-column appended to v (PV
    matmul emits [o; sum] in one accumulation group); the reciprocal is
    a single fast-approx DVE op, broadcast across partitions by the
    (otherwise idle) gpsimd engine.
  - v is projected directly into [j, dh] layout (x.T @ Wv with the
    x-tile stationary), so no PE transposes are needed.
"""

import numpy as np

from contextlib import ExitStack

import concourse.bass as bass
import concourse.bacc as bacc
import concourse.mybir as mybir
import concourse.tile as tile
from concourse.bass_utils import run_bass_kernel_spmd

F16 = mybir.dt.float16
F32 = mybir.dt.float32
F8 = mybir.dt.float8e4
DR = mybir.MatmulPerfMode.DoubleRow

# Problem shape (hardcoded; the harness always runs this config).
B, S, D = 2, 2048, 2048
H, HKV, DH = 32, 8, 64
WIN = 1024
SCALE = 1.0 / float(np.sqrt(DH))
MNEG = -30000.0             # pre-exp additive mask (exp -> 0 in f16)

N_CORES = 8
TP = 4                      # head-parallel ways
HLOC = H // TP              # 8 q heads per core
GLOC = HKV // TP            # 2 kv heads per core
EQ = HLOC * DH              # 512 q channels per core
EKV = GLOC * DH             # 128 kv channels per core


def _strip_taus(a, nstrip_t, wt):
    """j-tiles contributing to query strip a (4 i-tiles), with their
    valid column range inside the strip.  Returns list of
    (tau, c_lo, c_hi, is_diag, is_edge); a full-coverage tau is first so
    PSUM accumulation can start with a full 512-col write."""
    out = []
    for tau in range(max(0, 4 * a - wt), 4 * a + 4):
        t_lo = max(4 * a, tau)
        t_hi = min(4 * a + 3, tau + wt)
        if t_lo > t_hi or tau >= nstrip_t:
            continue
        c_lo = 128 * t_lo - 512 * a
        c_hi = 128 * (t_hi + 1) - 512 * a
        is_diag = 4 * a <= tau <= 4 * a + 3          # causal block at c_lo
        is_edge = (t_hi == tau + wt)                 # window-edge block at c_hi-128
        out.append((tau, c_lo, c_hi, is_diag, is_edge))
    full = [x for x in out if x[2] - x[1] == 512]
    assert full, f"strip {a} has no full-coverage tau"
    first = full[0]
    return [first] + [x for x in out if x is not first]


def _strip_chunks(a, nstrip_t, wt):
    """Group the strip's taus into score-PSUM chunks of <= 1024 columns
    (one 2-bank PSUM tile, one or two Exp activations).  Tau column
    ranges overlap in strip coordinates, so each tau gets a flattened
    offset `off` inside its chunk tile; no tau may straddle the 512-col
    PSUM bank boundary (matmul acc groups are bank-scoped), so offsets
    are packed into two 512-wide buckets.  Returns a list of
    (taus, ranges) where taus entries are
    (tau, c_lo, c_hi, is_diag, is_edge, off) and ranges lists the
    contiguous initialized column spans for the Exp."""
    taus = _strip_taus(a, nstrip_t, wt)
    chunks = []
    cur, used_a, used_b = [], 0, 0

    def close():
        nonlocal cur, used_a, used_b
        if cur:
            if used_a == 512 and used_b > 0:
                ranges = [(0, 512 + used_b)]
            else:
                ranges = [(0, used_a)]
                if used_b:
                    ranges.append((512, 512 + used_b))
            chunks.append((cur, ranges))
        cur, used_a, used_b = [], 0, 0

    for t in taus:
        w = t[2] - t[1]
        if used_a + w <= 512:
            off = used_a
            used_a += w
        elif used_b + w <= 512:
            off = 512 + used_b
            used_b += w
        else:
            close()
            off = 0
            used_a = w
        cur.append(t + (off,))
    close()
    return chunks


def build_program(s=S, d=D, win=WIN):
    """Emit the single-core SPMD program.  Returns nc."""
    nt = s // 128           # i/j tiles
    dc_n = d // 128         # contraction chunks for projections
    wt = win // 128
    nstrip = nt // 4

    nc = bacc.Bacc("TRN2", target_bir_lowering=False, debug=False,
                   num_devices=N_CORES)

    dram = {}

    def din(name, shape, dt):
        dram[name] = nc.dram_tensor(name, shape, dt, kind="ExternalInput").ap()
        return dram[name]

    # all big inputs are pre-laid-out on the host so each DMA reads long
    # per-partition contiguous runs (128 descriptors instead of 1000s)
    xtc = [din(f"xt{c}", [128, dc_n, 512], F16) for c in range(s // 512)]
    wq = din("wq", [128, 4, dc_n, 128], F16)  # et-major quarters
    wk = din("wk", [128, dc_n, EKV], F16)
    wv = din("wv", [128, dc_n, EKV], F16)
    wo = din("wo", [128, EQ // 128, d], F16)
    qaug = din("qaug", [2 * HLOC, s], F16)
    kaug = din("kaug", [2, s], F16)
    biascol = din("biascol", [128, 5], F32)   # q0..q3, k per-partition bias
    vbias = din("vbias", [1, EKV], F16)
    # packed [ident | mnlo | mnhi]: diag mask 0 if p<=f else MNEG,
    # edge mask 0 if p>f else MNEG
    cpk = din("cpk", [128, 384], F16)
    out_d = nc.dram_tensor("out", [s, d], F16, kind="ExternalOutput").ap()

    with tile.TileContext(nc) as tc, ExitStack() as ctx:
        P = ctx.enter_context
        consts = P(tc.tile_pool(name="consts", bufs=1))
        wpool = P(tc.tile_pool(name="wpool", bufs=1))
        xpool = P(tc.tile_pool(name="xpool", bufs=2))
        qapool = P(tc.tile_pool(name="qapool", bufs=1))
        vpool = P(tc.tile_pool(name="vpool", bufs=1))
        otpool = P(tc.tile_pool(name="otpool", bufs=1))
        wexp = P(tc.tile_pool(name="wexp", bufs=4))
        nrm = P(tc.tile_pool(name="nrm", bufs=2))
        osbp = P(tc.tile_pool(name="osbp", bufs=3))
        psX = P(tc.tile_pool(name="psX", bufs=2, space="PSUM"))
        psPV = P(tc.tile_pool(name="psPV", bufs=1, space="PSUM"))

        # ---- weights: wq split in quarters on the gpsimd SWDGE queue ----
        wq_sb = wpool.tile([128, 4, dc_n, 128], F16, name="wq_sb")
        for et in range(4):
            nc.gpsimd.dma_start(wq_sb[:, et, :, :], wq[:, et, :, :])
        wk_sb = wpool.tile([128, dc_n, EKV], F16, name="wk_sb")
        nc.gpsimd.dma_start(wk_sb[:], wk[:])
        wv_sb = wpool.tile([128, dc_n, EKV], F16, name="wv_sb")
        nc.gpsimd.dma_start(wv_sb[:], wv[:])
        # small consts on the scalar queue (idle at startup)
        cpk_sb = consts.tile([128, 384], F16, name="cpk_sb")
        nc.scalar.dma_start(cpk_sb[:], cpk[:])
        ident_sb = cpk_sb[:, 0:128]
        mnlo_sb = cpk_sb[:, 128:256]
        mnhi_sb = cpk_sb[:, 256:384]
        bias_sb = consts.tile([128, 5], F32, name="bias_sb")
        nc.scalar.dma_start(bias_sb[:], biascol[:])
        vbias_sb = consts.tile([1, EKV], F16, name="vbias_sb")
        nc.scalar.dma_start(vbias_sb[:], vbias[:])
        ones_col = consts.tile([1, 128], F16, name="ones_col")
        nc.vector.memset(ones_col[:], 1.0)
        # wo is first needed by the deferred output projection (after
        # attention strip 1) -- load it late so it doesn't eat startup
        # HBM bandwidth.  Emitted in the schedule loop below.
        wo_sb = wpool.tile([128, EQ // 128, d], F16, name="wo_sb")

        # ---- persistent activation tensors ----
        # score matmuls contract over partitions 0:66 only (64 channels
        # + 2 aug rows), so rows 66:128 need no zeroing.
        qa = []
        for h in range(HLOC):
            t = qapool.tile([128, s], F16, name=f"qa{h}")
            nc.gpsimd.dma_start(t[64:66, :], qaug[2 * h:2 * h + 2, :])
            qa.append(t)
        ka = []
        for g in range(GLOC):
            t = qapool.tile([128, s], F16, name=f"ka{g}")
            nc.gpsimd.dma_start(t[64:66, :], kaug[:, :])
            ka.append(t)
        va = []
        for g in range(GLOC):
            t = vpool.tile([128, nt, 65], F16, name=f"va{g}")
            nc.vector.memset(t[:, :, 64:65], 1.0)
            va.append(t)
        oT = []
        for ec in range(EQ // 128):
            t = otpool.tile([128, s], F16, name=f"oT{ec}")
            oT.append(t)

        # ---------- phase 1 emitter: projections for one s-chunk ----------
        # returns a list of piece-closures (one per PSUM-tile group) so
        # the scheduler can interleave projection work into attention
        # strips at pair granularity.
        def prep_proj_chunk(sc):
            xt = xpool.tile([128, dc_n, 512], F16, name="xt", tag="xt")
            if sc == 0:
                # startup-critical: per-k-tile DMAs alternating between
                # the sync and scalar queues, so the first accumulation
                # chain starts after 1/16 of the load
                for dc in range(dc_n):
                    eng = nc.scalar if dc % 2 else nc.sync
                    eng.dma_start(xt[:, dc:dc + 1, :],
                                  xtc[sc][:, dc:dc + 1, :])
            else:
                q4 = dc_n // 4
                for dq in range(4):
                    nc.sync.dma_start(
                        xt[:, dq * q4:(dq + 1) * q4, :],
                        xtc[sc][:, dq * q4:(dq + 1) * q4, :])
            cols = slice(sc * 512, (sc + 1) * 512)
            pieces = []

            def qk_piece(et):
                def run():
                    ps = psX.tile([128, 512], F32, name="ps_proj", tag="mm")
                    if et < EQ // 128:
                        w_lhs = lambda dc: wq_sb[:, et, dc, :]
                    else:
                        w_lhs = lambda dc: wk_sb[:, dc, :]
                    for dc in range(dc_n):
                        nc.tensor.matmul(ps[:], w_lhs(dc), xt[:, dc, :],
                                         start=(dc == 0), stop=(dc == dc_n - 1))
                    if et < EQ // 128:
                        nc.vector.tensor_scalar_add(
                            qa[2 * et][0:64, cols], ps[0:64, :],
                            bias_sb[0:64, et:et + 1])
                        nc.vector.tensor_scalar_add(
                            qa[2 * et + 1][0:64, cols], ps[64:128, :],
                            bias_sb[64:128, et:et + 1])
                    else:
                        nc.vector.tensor_scalar_add(
                            ka[0][0:64, cols], ps[0:64, :], bias_sb[0:64, 4:5])
                        nc.vector.tensor_scalar_add(
                            ka[1][0:64, cols], ps[64:128, :],
                            bias_sb[64:128, 4:5])
                return run

            def v_piece(jt):
                def run():
                    jg = sc * 4 + jt
                    psv = psX.tile([128, 512], F32, name="ps_v", tag="mm")
                    for dc in range(dc_n):
                        nc.tensor.matmul(
                            psv[:, 0:EKV],
                            xt[:, dc, jt * 128:(jt + 1) * 128],
                            wv_sb[:, dc, :],
                            start=(dc == 0), stop=False)
                    nc.tensor.matmul(psv[:, 0:EKV], ones_col[:], vbias_sb[:],
                                     start=False, stop=True)
                    nc.vector.tensor_copy(va[0][:, jg, 0:64], psv[:, 0:64])
                    nc.vector.tensor_copy(va[1][:, jg, 0:64], psv[:, 64:128])
                return run

            # k and v first: the next strip needs them for all its taus,
            # q columns are only read by that strip's own queries
            pieces.append(qk_piece(EQ // 128))        # k
            for jt in range(4):
                pieces.append(v_piece(jt))
            for et in range(EQ // 128):
                pieces.append(qk_piece(et))
            return pieces

        # ---------- phase 2 emitters ----------
        def emit_normalize(a, g, hp, pvs):
            # o[dh,i] = pv[dh,i] / pv[64,i]
            for u in range(2):
                h = g * 4 + hp * 2 + u
                dn = nrm.tile([1, 512], F32, name="dn", tag="dn")
                nc.scalar.copy(dn[:], pvs[u][64:65, :])
                rc = nrm.tile([1, 512], F32, name="rc", tag="rc")
                # custom-DVE ops read SBUF only -- dn must not be PSUM
                nc.vector.reciprocal_approx_fast(rc[:], dn[:])
                rcb = nrm.tile([64, 512], F32, name="rcb", tag="rcb")
                nc.gpsimd.partition_broadcast(rcb[:], rc[:], channels=64)
                r0 = (h % 2) * 64
                nc.vector.tensor_mul(
                    oT[h // 2][r0:r0 + 64, a * 512:(a + 1) * 512],
                    pvs[u][0:64, :], rcb[:])

        norm_pending = []   # deferred (a, g, hp, pvs)

        def flush_norms(keep=0):
            while len(norm_pending) > keep:
                emit_normalize(*norm_pending.pop(0))

        def emit_attn_pair(a, g, hp, chunks, flush_keep=1):
            pvs = []
            for u in range(2):
                pv = psPV.tile([128, 512], F32, name=f"pv{u}",
                               tag=f"pv{u}", bufs=2)
                pvs.append(pv)
            first_tau = chunks[0][0][0][0]
            last_tau = chunks[-1][0][-1][0]

            def drain_one(pend):
                ctaus, wts = pend.pop(0)
                for (tau, c_lo, c_hi, _d, _e, off) in ctaus:
                    for u in range(2):
                        nc.tensor.matmul(
                            pvs[u][0:65, c_lo:c_hi],
                            va[g][:, tau, :],
                            wts[u][:, off:off + c_hi - c_lo],
                            start=(tau == first_tau),
                            stop=(tau == last_tau and u == 1))

            # software pipeline: PV runs two chunks behind the scores so
            # the PE never waits on the Exp.
            pend = []        # [(chunk_taus, [w_u0, w_u1]), ...]
            first = True
            for (ctaus, ranges) in chunks:
                wts = []
                for u in range(2):
                    h = g * 4 + hp * 2 + u
                    ps = psX.tile([128, 1024], F32, name="ps_s", tag="mm")
                    for (tau, c_lo, c_hi, is_diag, is_edge, off) in ctaus:
                        ka_t = ka[g][0:66, tau * 128:(tau + 1) * 128]
                        lo, hi = off, off + c_hi - c_lo
                        qs = 512 * a + c_lo
                        if is_diag:
                            nc.tensor.matmul(
                                ps[:, lo:lo + 128], ka_t,
                                qa[h][0:66, qs:qs + 128],
                                start=True, stop=False)
                            nc.tensor.matmul(
                                ps[:, lo:lo + 128], ident_sb[:], mnlo_sb[:],
                                start=False, stop=True)
                            if hi > lo + 128:
                                nc.tensor.matmul(
                                    ps[:, lo + 128:hi], ka_t,
                                    qa[h][0:66, qs + 128:512 * a + c_hi],
                                    start=True, stop=True)
                        elif is_edge:
                            if hi - 128 > lo:
                                nc.tensor.matmul(
                                    ps[:, lo:hi - 128], ka_t,
                                    qa[h][0:66, qs:512 * a + c_hi - 128],
                                    start=True, stop=True)
                            nc.tensor.matmul(
                                ps[:, hi - 128:hi], ka_t,
                                qa[h][0:66,
                                      512 * a + c_hi - 128:512 * a + c_hi],
                                start=True, stop=False)
                            nc.tensor.matmul(
                                ps[:, hi - 128:hi], ident_sb[:], mnhi_sb[:],
                                start=False, stop=True)
                        else:
                            nc.tensor.matmul(
                                ps[:, lo:hi], ka_t,
                                qa[h][0:66, qs:512 * a + c_hi],
                                start=True, stop=True)
                    w_t = wexp.tile([128, 1024], F16, name=f"w{u}",
                                    tag=f"w{u}")
                    for (rlo, rhi) in ranges:
                        nc.scalar.activation(
                            w_t[:, rlo:rhi], ps[:, rlo:rhi],
                            mybir.ActivationFunctionType.Exp, scale=SCALE)
                    wts.append(w_t)
                if len(pend) >= 2:
                    drain_one(pend)
                if first:
                    # older pairs' normalizes hide under this pair's work;
                    # at a strip's first pair everything must drain so the
                    # interleaved oproj below reads finished oT rows
                    flush_norms(keep=flush_keep)
                    first = False
                pend.append((ctaus, wts))
            while pend:
                drain_one(pend)
            norm_pending.append((a, g, hp, pvs))

        def emit_oproj_st(st):
            for db in range(d // 1024):
                ps = psX.tile([128, 1024], F32, name="ps_o", tag="mm")
                for half in range(2):
                    dcb = db * 2 + half
                    for ec in range(EQ // 128):
                        nc.tensor.matmul(
                            ps[:, half * 512:(half + 1) * 512],
                            oT[ec][:, st * 128:(st + 1) * 128],
                            wo_sb[:, ec, dcb * 512:(dcb + 1) * 512],
                            start=(ec == 0), stop=(ec == EQ // 128 - 1))
                osb = osbp.tile([128, 1024], F16, name="osb", tag="osb")
                if (st + db) % 2 == 0:
                    nc.vector.tensor_copy(osb[:], ps[:])
                else:
                    nc.scalar.copy(osb[:], ps[:])
                nc.sync.dma_start(
                    out_d[st * 128:(st + 1) * 128,
                          db * 1024:(db + 1) * 1024], osb[:])

        def emit_attn_strip(a, oproj_strip=None, filler=()):
            # oproj of a previous strip and the next strip's projection
            # pieces are interleaved at pair granularity: pure-PE work
            # fills the pipe while this strip's Exps catch up, so the
            # Act engine is never the pacing engine.
            chunks = _strip_chunks(a, nt, wt)
            filler = list(filler)
            pi = 0
            for g in range(GLOC):
                for hp in range(2):
                    emit_attn_pair(a, g, hp, chunks,
                                   flush_keep=0 if pi == 0 else 1)
                    if oproj_strip is not None:
                        emit_oproj_st(4 * oproj_strip + pi)
                    take = (len(filler) + 3 - pi) // (4 - pi)
                    for _ in range(take):
                        filler.pop(0)()
                    pi += 1
            for p in filler:
                p()

        # ---------- schedule ----------
        # attention strip a only needs proj chunks <= a: chunk 0 runs up
        # front, chunk a+1 spreads across strip a.
        for p in prep_proj_chunk(0):
            p()
        for a in range(nstrip):
            if a == 1:
                # past the startup DMA crunch; wo needed after this strip
                nc.gpsimd.dma_start(wo_sb[:], wo[:])
            nxt = prep_proj_chunk(a + 1) if a + 1 < nstrip else []
            emit_attn_strip(a, oproj_strip=a - 1 if a > 0 else None,
                            filler=nxt)
        flush_norms()
        for st in range(4 * (nstrip - 1), 4 * nstrip):
            emit_oproj_st(st)

    nc.compile()
    return nc


# ---------------- host-side sharding ----------------

def _tile_pce(a2d):
    """[C*128, E] -> [128, C, E]: partition-major layout for clean DMA."""
    cc, e = a2d.shape[0] // 128, a2d.shape[1]
    return np.ascontiguousarray(
        np.transpose(a2d.reshape(cc, 128, e), (1, 0, 2)))


def _fp8_splits(a2d_f32):
    """[C*128, E] f32 -> (hi, lo, h64) each [128, C//2, 2, E] fp8 for the
    compensated DoubleRow projection: hi = fp8(a), lo = fp8(a - hi),
    h64 = fp8(hi / 64) (pairs with 64*Wlo on the other operand)."""
    import ml_dtypes
    f8 = ml_dtypes.float8_e4m3
    cc, e = a2d_f32.shape[0] // 128, a2d_f32.shape[1]

    def pairs(a):
        t = np.transpose(a.reshape(cc, 128, e), (1, 0, 2))
        return np.ascontiguousarray(t.reshape(128, cc // 2, 2, e))

    hi = a2d_f32.astype(f8)
    lo = (a2d_f32 - hi.astype(np.float32)).astype(f8)
    h64 = (hi.astype(np.float32) / 64.0).astype(f8)
    return pairs(hi), pairs(lo), pairs(h64)


def _prep_core_inputs(c, x, Wq, bq, Wk, bk, Wv, bv, Wo, slopes, s=S, d=D):
    """Build the per-core input map (all numpy, fp16 where declared)."""
    b = c // TP
    hs = c % TP
    f16 = np.float16
    qrows = slice(hs * EQ, (hs + 1) * EQ)
    krows = slice(hs * EKV, (hs + 1) * EKV)
    m = {}
    xT = x[b].T.astype(f16)                       # [d, s]
    for sc in range(s // 512):
        m[f"xt{sc}"] = _tile_pce(xT[:, sc * 512:(sc + 1) * 512])
    wq3 = _tile_pce(np.ascontiguousarray(Wq[qrows, :].T).astype(f16))
    m["wq"] = np.ascontiguousarray(
        np.transpose(wq3.reshape(128, 16, 4, 128), (0, 2, 1, 3)))
    m["wk"] = _tile_pce(np.ascontiguousarray(Wk[krows, :].T).astype(f16))
    m["wv"] = _tile_pce(np.ascontiguousarray(Wv[krows, :].T).astype(f16))
    m["wo"] = _tile_pce(np.ascontiguousarray(Wo[:, qrows].T).astype(f16))
    qaug = np.zeros((2 * HLOC, s), np.float32)
    i_idx = np.arange(s, dtype=np.float32)
    for h in range(HLOC):
        sl = float(slopes[hs * HLOC + h])
        qaug[2 * h, :] = sl / SCALE
        qaug[2 * h + 1, :] = -sl / SCALE * i_idx
    m["qaug"] = qaug.astype(f16)
    kaug = np.zeros((2, s), np.float32)
    kaug[0, :] = i_idx
    kaug[1, :] = 1.0
    m["kaug"] = kaug.astype(f16)
    # per-partition bias columns: cols 0..3 q et-blocks, col 4 k
    bcol = np.zeros((128, 5), np.float32)
    for et in range(4):
        bcol[:, et] = bq[qrows][et * 128:(et + 1) * 128]
    bcol[:, 4] = bk[krows]
    m["biascol"] = bcol
    m["vbias"] = bv[krows].astype(f16).reshape(1, -1)
    p = np.arange(128)[:, None]
    f = np.arange(128)[None, :]
    m["cpk"] = np.concatenate([
        np.eye(128, dtype=f16),
        np.where(p <= f, 0.0, MNEG).astype(f16),
        np.where(p > f, 0.0, MNEG).astype(f16)], axis=1)
    return m


_PROG_CACHE = {}


def _get_program():
    key = (S, D, WIN)
    if key not in _PROG_CACHE:
        _PROG_CACHE[key] = build_program()
    return _PROG_CACHE[key]


def kernel(hidden_states, Wq, bq, Wk, bk, Wv, bv, Wo, bo, alibi_slopes,
           _want_profile=False):
    x = np.asarray(hidden_states, np.float32)
    Wq = np.asarray(Wq, np.float32)
    Wk = np.asarray(Wk, np.float32)
    Wv = np.asarray(Wv, np.float32)
    Wo = np.asarray(Wo, np.float32)
    bq = np.asarray(bq, np.float32)
    bk = np.asarray(bk, np.float32)
    bv = np.asarray(bv, np.float32)
    bo = np.asarray(bo, np.float32)
    slopes = np.asarray(alibi_slopes, np.float32)

    nc = _get_program()
    in_maps = [
        _prep_core_inputs(c, x, Wq, bq, Wk, bk, Wv, bv, Wo, slopes)
        for c in range(N_CORES)
    ]
    res = run_bass_kernel_spmd(nc, in_maps, list(range(N_CORES)),
                               trace=_want_profile)
    out = np.zeros((B, S, D), np.float32)
    for c in range(N_CORES):
        out[c // TP] += res.results[c]["out"].astype(np.float32)
    out += bo[None, None, :]
    if _want_profile:
        return out, res
    return out
